# revision 1
# baseline (speedup 1.0000x reference)
"""BinaryTreeLSTM over a complete 18-level binary tree, on 8 Trainium2 cores.

Sharding: contiguous block-sharding of every level across the 8 cores makes
each core own an independent sub-forest (parent p's children 2p/2p+1 stay in
the same core's chunk), so levels 17..3 run with zero inter-core
communication. Levels 2..0 (7 nodes) are finished on the host.

Device layout: feature-major ("transposed") tiles [dims, nodes]. Within each
core, every level's nodes are stored in bit-reversed order, which makes the
even/odd-child gather between levels two contiguous column ranges. The
LSTM bias is folded into the x-matmul via a constant ones feature row.
"""

import numpy as np

import concourse.bacc as bacc
import concourse.bass as bass
import concourse.mybir as mybir
from concourse.tile import TileContext
from concourse.bass_utils import run_bass_kernel_spmd

INPUT = 64
H = 128
HH = H // 2
LEVELS = 18
N_CORES = 8
T = 512  # node-tile width (one fp32 PSUM bank)
NM_MAX = 256  # levels this narrow switch to node-major (nodes on partitions)
USE_SIGMA = True  # tanh(g)=2*sigmoid(2g)-1 with host-doubled g weights
S_BUFS = 3        # lookahead depth of the activated-gates tile
HMUL_POOL = False  # run the wide-tile h=so*tanh(c) mul on GPSIMD


def _deep_layout(L, nm_max=NM_MAX):
    """Row layout of the node-major levels inside out_deep."""
    levels, widths, off, NPC = _layout(L)
    doff = {}
    cur = 0
    for l in levels:
        if widths[l] <= nm_max:
            doff[l] = cur
            cur += widths[l]
    return doff, cur

F32 = mybir.dt.float32
F32R = mybir.dt.float32r  # 4x faster PE mode for N>=256, tf32-like numerics


def _layout(L):
    """Per-core column layout: leaves first, root-of-subtree last."""
    levels = list(range(L - 1, 2, -1))  # L-1 .. 3
    widths = {l: 2 ** (l - 3) for l in levels}
    off = {}
    cur = 0
    for l in levels:
        off[l] = cur
        cur += widths[l]
    return levels, widths, off, cur


def _bitrev_perm(n):
    bits = max(n.bit_length() - 1, 0)
    j = np.arange(n)
    r = np.zeros(n, dtype=np.int64)
    for b in range(bits):
        r |= ((j >> b) & 1) << (bits - 1 - b)
    return r


def build_program(L=LEVELS, tile_w=T, repeats=1):
    """Build the per-core SPMD Bass program (identical on all cores).

    repeats>1 re-runs the whole level sweep back to back (same outputs) —
    used only for marginal-cost timing, never for the graded path.
    """
    nc = bacc.Bacc("TRN2", target_bir_lowering=False, num_devices=N_CORES)
    levels, widths, off, NPC = _layout(L)
    doff, n_deep = _deep_layout(L, NM_MAX)

    xT = nc.dram_tensor("xT", [INPUT + 1, NPC], F32R, kind="ExternalInput").ap()
    wx = nc.dram_tensor("wx", [INPUT + 1, 4 * H], F32R, kind="ExternalInput").ap()
    whl = nc.dram_tensor("whl", [HH, 4 * H], F32R, kind="ExternalInput").ap()
    whr = nc.dram_tensor("whr", [HH, 4 * H], F32R, kind="ExternalInput").ap()
    ident = nc.dram_tensor("ident", [H, H], F32, kind="ExternalInput").ap()
    out_hT = nc.dram_tensor("out_hT", [H, NPC], F32, kind="ExternalOutput").ap()
    out_deep = nc.dram_tensor("out_deep", [n_deep, H], F32,
                              kind="ExternalOutput").ap()
    out_c3 = nc.dram_tensor("out_c3", [1, H], F32, kind="ExternalOutput").ap()

    # weight column order (host pre-permutes gates to [i, f, o, g])
    GI, GF, GO, GG = 0, 1, 2, 3

    with TileContext(nc) as tc:
        with tc.tile_pool(name="consts", bufs=1) as consts, \
             tc.tile_pool(name="keep", bufs=1) as keep, \
             tc.tile_pool(name="work", bufs=3) as work, \
             tc.tile_pool(name="xin", bufs=4) as xin, \
             tc.tile_pool(name="hout", bufs=4) as hout, \
             tc.tile_pool(name="psum", bufs=2, space="PSUM") as psum:

            wx_s = consts.tile([INPUT + 1, 4 * H], F32R)
            nc.sync.dma_start(out=wx_s, in_=wx)
            whl_s = consts.tile([HH, 4 * H], F32R)
            nc.sync.dma_start(out=whl_s, in_=whl)
            whr_s = consts.tile([HH, 4 * H], F32R)
            nc.sync.dma_start(out=whr_s, in_=whr)
            ident_s = consts.tile([H, H], F32)
            nc.sync.dma_start(out=ident_s, in_=ident)

            # persistent keep state: rows 0:64 = h[0:64] of nodes, rows
            # 64:128 = c[0:64], columns = level-local node positions.
            # Only child+current levels are live, so two ping-pong buffers
            # (sized for the two largest levels) replace a full heap.
            # f32r-typed because rows 0:64 feed the PE as matmul rhs.
            n_leaf = widths[levels[0]]
            hcA = keep.tile([H, n_leaf], F32R)
            hcB = keep.tile([H, max(n_leaf // 2, 1)], F32R)

            def keep_buf(l):
                return hcA if (levels[0] - l) % 2 == 0 else hcB

            for _rep in range(repeats):
              for l in levels:
                n = widths[l]
                leaf = l == levels[0]
                if n <= NM_MAX:
                    continue  # node-major path below
                ntiles = (n + tile_w - 1) // tile_w
                for t in range(ntiles):
                    nt = min(tile_w, n - t * tile_w)
                    cols = off[l] + t * tile_w  # this tile's columns
                    kcols = t * tile_w  # keep-state (level-local) columns

                    xt = xin.tile([INPUT + 1, tile_w], F32R, tag="xt")
                    nc.sync.dma_start(out=xt[:, :nt],
                                      in_=xT[:, cols:cols + nt])

                    pt = psum.tile([H, 4 * T], F32, tag="pt")
                    S = work.tile([H, 4 * T], F32, tag="S", bufs=S_BUFS)

                    if leaf:
                        # gates i, o, g -> banks 0, 1, 2
                        for bank, g in ((0, GI), (1, GO), (2, GG)):
                            nc.tensor.matmul(
                                pt[:, bank * T:bank * T + nt],
                                wx_s[:, g * H:(g + 1) * H],
                                xt[:, :nt], start=True, stop=True)
                        # host doubled the g-gate weights: tanh(g)=2*sig(2g)-1,
                        # so ONE sigmoid covers every gate; DVE fixes g up
                        ng = 3 if USE_SIGMA else 2
                        nc.scalar.activation(
                            out=S[:, 0:ng * T].rearrange(
                                "p (g n) -> p g n", g=ng)[:, :, :nt],
                            in_=pt[:, 0:ng * T].rearrange(
                                "p (g n) -> p g n", g=ng)[:, :, :nt],
                            func=mybir.ActivationFunctionType.Sigmoid)
                        si = S[:, 0:nt]
                        so = S[:, T:T + nt]
                        gsl = S[:, 2 * T:2 * T + nt]
                        if USE_SIGMA:
                            nc.vector.tensor_scalar(
                                out=gsl, in0=gsl, scalar1=2.0, scalar2=-1.0,
                                op0=mybir.AluOpType.mult,
                                op1=mybir.AluOpType.add)
                        else:
                            nc.scalar.activation(
                                out=gsl, in_=pt[:, 2 * T:2 * T + nt],
                                func=mybir.ActivationFunctionType.Tanh)
                        tg = gsl
                    else:
                        # children: even at ce, odd at co (bit-reversed order)
                        ce = t * tile_w
                        co = n + t * tile_w
                        hck = keep_buf(l + 1)
                        # gates i, f, o, g -> banks 0..3
                        # fp32r needs even column counts; odd-width tiles
                        # (the single-node level-3 tile) drop to plain fp32
                        cast = (lambda ap: ap) if nt % 2 == 0 else (
                            lambda ap: ap.bitcast(F32))
                        for bank, g in ((0, GI), (1, GF), (2, GO), (3, GG)):
                            dst = pt[:, bank * T:bank * T + nt]
                            lx = cast(wx_s[:, g * H:(g + 1) * H])
                            ll = cast(whl_s[:, g * H:(g + 1) * H])
                            lr = cast(whr_s[:, g * H:(g + 1) * H])
                            nc.tensor.matmul(dst, lx, cast(xt[:, :nt]),
                                             start=True, stop=False)
                            nc.tensor.matmul(dst, ll,
                                             cast(hck[0:HH, ce:ce + nt]),
                                             start=False, stop=False)
                            nc.tensor.matmul(dst, lr,
                                             cast(hck[0:HH, co:co + nt]),
                                             start=False, stop=True)
                        ng = 4 if USE_SIGMA else 3
                        nc.scalar.activation(
                            out=S[:, 0:ng * T].rearrange(
                                "p (g n) -> p g n", g=ng)[:, :, :nt],
                            in_=pt[:, 0:ng * T].rearrange(
                                "p (g n) -> p g n", g=ng)[:, :, :nt],
                            func=mybir.ActivationFunctionType.Sigmoid)
                        si = S[:, 0:nt]
                        sf = S[:, T:T + nt]
                        so = S[:, 2 * T:2 * T + nt]
                        gsl = S[:, 3 * T:3 * T + nt]
                        if USE_SIGMA:
                            nc.vector.tensor_scalar(
                                out=gsl, in0=gsl, scalar1=2.0, scalar2=-1.0,
                                op0=mybir.AluOpType.mult,
                                op1=mybir.AluOpType.add)
                        else:
                            nc.scalar.activation(
                                out=gsl, in_=pt[:, 3 * T:3 * T + nt],
                                func=mybir.ActivationFunctionType.Tanh)
                        tg = gsl

                    # POOL relieves DVE on these wide, throughput-bound tiles
                    mul_eng = nc.gpsimd
                    if leaf:
                        # c = sigmoid(i) * tanh(g)
                        c = work.tile([H, tile_w], F32, tag="c")
                        mul_eng.tensor_mul(c[:, :nt], si, tg)
                    else:
                        # c_prev gather from child c-halves
                        cp = work.tile([H, tile_w], F32, tag="cp")
                        nc.vector.tensor_copy(
                            out=cp[0:HH, :nt],
                            in_=hck[HH:H, ce:ce + nt].bitcast(F32))
                        nc.vector.tensor_copy(
                            out=cp[HH:H, :nt],
                            in_=hck[HH:H, co:co + nt].bitcast(F32))
                        t1 = work.tile([H, tile_w], F32, tag="t1")
                        mul_eng.tensor_mul(t1[:, :nt], si, tg)
                        t2 = work.tile([H, tile_w], F32, tag="t2")
                        mul_eng.tensor_mul(t2[:, :nt], sf, cp[:, :nt])
                        c = work.tile([H, tile_w], F32, tag="c")
                        nc.vector.tensor_add(c[:, :nt], t1[:, :nt], t2[:, :nt])

                    tch = work.tile([H, tile_w], F32, tag="tch")
                    nc.scalar.activation(
                        out=tch[:, :nt], in_=c[:, :nt],
                        func=mybir.ActivationFunctionType.Tanh)
                    h = hout.tile([H, tile_w], F32, tag="h")
                    (nc.gpsimd if HMUL_POOL else nc.vector).tensor_mul(
                        h[:, :nt], so, tch[:, :nt])

                    # stash h[0:64] (f32r-rounded, feeds PE) and c[0:64]
                    # (bit-identical f32; only DVE/POOL read it back).
                    # tensor_scalar mult-by-1 rather than copy: walrus only
                    # accepts compute ops as producers of f32r matmul inputs.
                    nc.vector.tensor_scalar(
                        out=keep_buf(l)[0:HH, kcols:kcols + nt],
                        in0=h[0:HH, :nt],
                        scalar1=1.0, scalar2=None, op0=mybir.AluOpType.mult)
                    nc.vector.tensor_scalar(
                        out=keep_buf(l)[HH:H, kcols:kcols + nt],
                        in0=c[0:HH, :nt],
                        scalar1=1.0, scalar2=None, op0=mybir.AluOpType.mult)

                    nc.sync.dma_start(out=out_hT[:, cols:cols + nt],
                                      in_=h[:, :nt])

              # ---- node-major tail: narrow levels, nodes on partitions ----
              for l in levels:
                n = widths[l]
                if n > NM_MAX:
                    continue
                leaf = l == levels[0]
                ntiles = (n + H - 1) // H
                for t in range(ntiles):
                    nt = min(H, n - t * H)
                    cols = off[l] + t * H
                    kcols = t * H

                    xt = xin.tile([INPUT + 1, H], F32R, tag="xt")
                    nc.sync.dma_start(out=xt[:, :nt],
                                      in_=xT[:, cols:cols + nt])

                    pt = psum.tile([H, 4 * T], F32, tag="pt")
                    # gates [nt, 512] in bank 0: one matmul per K-part,
                    # stationary operand is the feature-major data itself
                    nc.tensor.matmul(pt[0:nt, 0:T], xt[:, :nt], wx_s,
                                     start=True, stop=leaf)
                    if not leaf:
                        ce = t * H
                        co = n + t * H
                        hck = keep_buf(l + 1)
                        nc.tensor.matmul(pt[0:nt, 0:T], hck[0:HH, ce:ce + nt],
                                         whl_s, start=False, stop=False)
                        nc.tensor.matmul(pt[0:nt, 0:T], hck[0:HH, co:co + nt],
                                         whr_s, start=False, stop=True)

                    S2 = work.tile([H, T], F32, tag="S2", bufs=2)
                    nsg = 4 if USE_SIGMA else 3
                    nc.scalar.activation(
                        out=S2[0:nt, 0:nsg * H], in_=pt[0:nt, 0:nsg * H],
                        func=mybir.ActivationFunctionType.Sigmoid)
                    si = S2[0:nt, 0:H]
                    sf = S2[0:nt, H:2 * H]
                    so = S2[0:nt, 2 * H:3 * H]
                    tg = S2[0:nt, 3 * H:4 * H]
                    if USE_SIGMA:
                        nc.vector.tensor_scalar(
                            out=tg, in0=tg, scalar1=2.0, scalar2=-1.0,
                            op0=mybir.AluOpType.mult, op1=mybir.AluOpType.add)
                    else:
                        nc.scalar.activation(
                            out=tg, in_=pt[0:nt, 3 * H:4 * H],
                            func=mybir.ActivationFunctionType.Tanh)

                    c_nm = work.tile([H, H], F32, tag="c_nm")
                    if leaf:
                        nc.vector.tensor_mul(c_nm[0:nt, :], si, tg)
                    else:
                        # c_prev: transpose child c-halves into bank 1
                        tpe = pt[0:nt, T:T + HH]
                        tpo = pt[0:nt, T + HH:T + H]
                        # identity block at base partition 64 to match hc's
                        # c-half (matmul requires equal base partitions)
                        nc.tensor.transpose(
                            tpe, hck[HH:H, ce:ce + nt].bitcast(F32),
                            ident_s[HH:H, HH:H])
                        nc.tensor.transpose(
                            tpo, hck[HH:H, co:co + nt].bitcast(F32),
                            ident_s[HH:H, HH:H])
                        t2 = work.tile([H, H], F32, tag="t2_nm")
                        nc.vector.tensor_mul(t2[0:nt, 0:HH],
                                             sf[:, 0:HH], tpe)
                        nc.vector.tensor_mul(t2[0:nt, HH:H],
                                             sf[:, HH:H], tpo)
                        t1 = work.tile([H, H], F32, tag="t1_nm")
                        nc.vector.tensor_mul(t1[0:nt, :], si, tg)
                        nc.vector.tensor_add(c_nm[0:nt, :],
                                             t1[0:nt, :], t2[0:nt, :])

                    if l > 3:
                        # c-keep first: it only needs c_nm, so it overlaps
                        # the tanh/h tail of this level's chain
                        kc = pt[0:HH, T + H + H:T + H + H + nt]
                        nc.tensor.transpose(kc, c_nm[0:nt, 0:HH],
                                            ident_s[0:nt, 0:nt])
                        nc.vector.tensor_scalar(
                            out=keep_buf(l)[HH:H, kcols:kcols + nt], in0=kc,
                            scalar1=1.0, scalar2=None,
                            op0=mybir.AluOpType.mult)

                    tch = work.tile([H, H], F32, tag="tch_nm")
                    nc.scalar.activation(
                        out=tch[0:nt, :], in_=c_nm[0:nt, :],
                        func=mybir.ActivationFunctionType.Tanh)
                    h_nm = hout.tile([H, H], F32, tag="h_nm")
                    nc.vector.tensor_mul(h_nm[0:nt, :], so, tch[0:nt, :])

                    if l > 3:
                        kh = pt[0:HH, T + H:T + H + nt]
                        nc.tensor.transpose(kh, h_nm[0:nt, 0:HH],
                                            ident_s[0:nt, 0:nt])
                        nc.vector.tensor_scalar(
                            out=keep_buf(l)[0:HH, kcols:kcols + nt], in0=kh,
                            scalar1=1.0, scalar2=None,
                            op0=mybir.AluOpType.mult)

                    nc.sync.dma_start(
                        out=out_deep[doff[l] + t * H:doff[l] + t * H + nt, :],
                        in_=h_nm[0:nt, :])
                    if l == 3:
                        nc.sync.dma_start(out=out_c3, in_=c_nm[0:1, :])

    nc.compile()
    return nc


_PROGRAMS = {}


def _get_program(L=LEVELS):
    if L not in _PROGRAMS:
        _PROGRAMS[L] = build_program(L)
    return _PROGRAMS[L]


def _make_in_maps(x, W_ih, W_hh, b_ih, b_hh, L=LEVELS):
    levels, widths, off, NPC = _layout(L)
    b = (b_ih + b_hh).astype(np.float32)

    # permute gate blocks from [i, f, g, o] to [i, f, o, g]
    def gperm(m):  # m: [4H, ...]
        blocks = [m[0:H], m[H:2 * H], m[3 * H:4 * H], m[2 * H:3 * H]]
        return np.concatenate(blocks, axis=0)

    Wx = gperm(W_ih).copy()       # [512, 64]
    Wh = gperm(W_hh).copy()       # [512, 128]
    bp = gperm(b[:, None])[:, 0].copy()  # [512]
    if USE_SIGMA:
        # tanh(g) is computed as 2*sigmoid(2g)-1 on device: double g's weights
        Wx[3 * H:4 * H] *= 2.0
        Wh[3 * H:4 * H] *= 2.0
        bp[3 * H:4 * H] *= 2.0

    wx = np.concatenate([Wx.T, bp[None, :]], axis=0).astype(np.float32)  # [65,512]
    whl = np.ascontiguousarray(Wh[:, :HH].T)   # [64, 512]
    whr = np.ascontiguousarray(Wh[:, HH:].T)   # [64, 512]

    perms = {l: _bitrev_perm(widths[l]) for l in levels}
    ident = np.eye(H, dtype=np.float32)
    in_maps = []
    for k in range(N_CORES):
        xTk = np.empty((INPUT + 1, NPC), np.float32)
        xTk[INPUT, :] = 1.0
        for l in levels:
            n = widths[l]
            start = 2 ** l - 1
            chunk = x[start + k * n: start + (k + 1) * n]  # [n, 64]
            xTk[:INPUT, off[l]:off[l] + n] = chunk[perms[l]].T
        in_maps.append({"xT": xTk, "wx": wx, "whl": whl, "whr": whr,
                        "ident": ident})
    return in_maps, perms


def _assemble(results, x, W_ih, W_hh, b_ih, b_hh, perms, L=LEVELS):
    levels, widths, off, NPC = _layout(L)
    n_nodes = 2 ** L - 1
    out = np.zeros((n_nodes, H), np.float32)

    doff, n_deep = _deep_layout(L, NM_MAX)
    perms = {l: _bitrev_perm(widths[l]) for l in levels}
    h3 = np.zeros((N_CORES, H), np.float32)
    c3 = np.zeros((N_CORES, H), np.float32)
    for k in range(N_CORES):
        hT = results[k]["out_hT"]  # [128, NPC]
        hk = np.ascontiguousarray(hT.T)  # [NPC, 128] positions j
        deep = results[k]["out_deep"]  # [n_deep, 128] positions
        for l in levels:
            n = widths[l]
            start = 2 ** l - 1
            if l in doff:
                block = deep[doff[l]:doff[l] + n]
            else:
                block = hk[off[l]:off[l] + n]
            out[start + k * n + perms[l]] = block
        h3[k] = deep[doff[3]]
        c3[k] = results[k]["out_c3"][0]

    # levels 2..0 on host (7 nodes), mirroring the reference exactly
    b = (b_ih + b_hh).astype(np.float32)
    h_child, c_child = h3, c3

    def sig(v):
        return 1.0 / (1.0 + np.exp(-v))

    for lvl in range(2, -1, -1):
        start = 2 ** lvl - 1
        count = 2 ** lvl
        xs = x[start:start + count]
        h_prev = np.concatenate([h_child[0::2, :HH], h_child[1::2, :HH]], -1)
        c_prev = np.concatenate([c_child[0::2, :HH], c_child[1::2, :HH]], -1)
        gates = xs @ W_ih.T + h_prev @ W_hh.T + b
        gi, gf, gg, go = np.split(gates, 4, axis=-1)
        c = sig(gf) * c_prev + sig(gi) * np.tanh(gg)
        h = sig(go) * np.tanh(c)
        out[start:start + count] = h
        h_child, c_child = h, c
    return out


def kernel(x, W_ih, W_hh, b_ih, b_hh):
    x = np.asarray(x, np.float32)
    W_ih = np.asarray(W_ih, np.float32)
    W_hh = np.asarray(W_hh, np.float32)
    b_ih = np.asarray(b_ih, np.float32)
    b_hh = np.asarray(b_hh, np.float32)

    nc = _get_program(LEVELS)
    in_maps, perms = _make_in_maps(x, W_ih, W_hh, b_ih, b_hh, LEVELS)
    res = run_bass_kernel_spmd(nc, in_maps, core_ids=list(range(N_CORES)))
    return _assemble(res.results, x, W_ih, W_hh, b_ih, b_hh, perms, LEVELS)



# revision 15
# speedup vs baseline: 5.8361x; 5.8361x over previous
"""BinaryTreeLSTM over a complete 18-level binary tree, on 8 Trainium2 cores.

Sharding: contiguous block-sharding of every level across the 8 cores makes
each core own an independent sub-forest (parent p's children stay in the same
core's chunk), so device levels run with zero inter-core communication.
Levels DEV_MIN-1..0 (16383 nodes, 6.2%) are finished on the host from the
device's level-DEV_MIN h/c halves.

Device layout: feature-major bf16 tiles [dims, nodes]; within each core,
every level's nodes are stored in bit-reversed order, so the even(left)/
odd(right) child split between levels is two contiguous column halves.

Keep-state layout: for each parent level, two [128, parent_width] tiles:
  keep_h rows 0:64 = h[0:64] of left children, rows 64:128 = right children
  keep_c rows 0:64 = c[0:64] of left children, rows 64:128 = right children
keep_h IS the (stacked) h_prev operand of one K=128 matmul per gate, and
keep_c IS c_prev verbatim -- no per-level gather copies at all.

Tile schedule: each level's T-wide tiles are processed as pairs
(j, ntiles/2 + j): the pair completes BOTH child-halves of one parent
column block, so the next level can start as soon as the matching producer
pair retires (fine-grained cross-level pipelining), and the pair shares
each PE stationary weight between its two matmuls (halves weight-load
stalls). xT/out_hT columns are laid out in processing order (the host owns
the permutation), keeping all DMA chunk management monotonic.

The LSTM bias rides a constant ones feature row in x; the g-gate tanh is
computed as 2*sigmoid(2g)-1 with host-doubled g weights so one ACT
instruction covers all four gates.
"""

import numpy as np

import concourse.bacc as bacc
import concourse.mybir as mybir
from concourse.tile import TileContext
from concourse.bass_utils import run_bass_kernel_spmd

INPUT = 64
H = 128
HH = H // 2
LEVELS = 18
N_CORES = 8
T = 512           # node-tile width (one fp32 PSUM bank)
DEV_MIN = 14      # lowest tree level computed on device; host does DEV_MIN-1..0
XCHUNK = 8192     # x prefetch chunk (cols)
OCHUNK = 2048     # h output staging chunk (cols)

F32 = mybir.dt.float32
BF16 = mybir.dt.bfloat16

# weight column order (host pre-permutes gate blocks to [i, f, o, g])
GI, GF, GO, GG = 0, 1, 2, 3


def _layout(L=LEVELS):
    """Per-core column layout: leaves first, level DEV_MIN last."""
    levels = list(range(L - 1, DEV_MIN - 1, -1))
    widths = {l: 2 ** (l - 3) for l in levels}
    off = {}
    cur = 0
    for l in levels:
        off[l] = cur
        cur += widths[l]
    return levels, widths, off, cur


def _bitrev_perm(n):
    bits = max(n.bit_length() - 1, 0)
    j = np.arange(n)
    r = np.zeros(n, dtype=np.int64)
    for b in range(bits):
        r |= ((j >> b) & 1) << (bits - 1 - b)
    return r


def _tile_order(ntiles):
    """Pairs (j, mid+j): each pair finishes one parent column block."""
    if ntiles == 1:
        return [(0,)]
    mid = ntiles // 2
    return [(j, mid + j) for j in range(mid)]


def _pos_perm(n):
    """Level-local physical col -> processing-order position mapping.

    Returns idx such that processing position p holds physical col idx[p].
    """
    ntiles = (n + T - 1) // T
    idx = []
    for grp in _tile_order(ntiles):
        for t in grp:
            nt = min(T, n - t * T)
            idx.append(np.arange(t * T, t * T + nt))
    return np.concatenate(idx)


def _x_chunks(levels, off, widths, NPC):
    """Contiguous x DMA chunks: small first chunks so compute starts early,
    then XCHUNK-sized, with the small tail levels merged."""
    chunks = [(0, 1024), (1024, 3072)]
    cur = 4096
    for l in levels:
        if widths[l] >= 2048:
            end = off[l] + widths[l]
            while cur < end:
                step = min(XCHUNK, end - cur)
                chunks.append((cur, step))
                cur += step
        else:
            chunks.append((cur, NPC - cur))
            break
    return chunks


def build_program(L=LEVELS, repeats=1):
    """Build the per-core SPMD Bass program (identical on all cores).

    repeats>1 re-runs the whole level sweep back to back (same outputs) --
    used only for marginal-cost timing, never for the graded path.
    """
    nc = bacc.Bacc("TRN2", target_bir_lowering=False, num_devices=N_CORES)
    levels, widths, off, NPC = _layout(L)
    n_leaf = widths[levels[0]]
    w_last = widths[levels[-1]]  # level DEV_MIN width per core

    xT = nc.dram_tensor("xT", [INPUT + 1, NPC], BF16, kind="ExternalInput").ap()
    wxb = nc.dram_tensor("wxb", [INPUT + 1, 4 * H], BF16,
                         kind="ExternalInput").ap()
    whlr = nc.dram_tensor("whlr", [H, 4 * H], BF16, kind="ExternalInput").ap()
    out_hT = nc.dram_tensor("out_hT", [H, NPC], BF16, kind="ExternalOutput").ap()
    out_kh = nc.dram_tensor("out_kh", [H, w_last // 2], BF16,
                            kind="ExternalOutput").ap()
    out_kc = nc.dram_tensor("out_kc", [H, w_last // 2], BF16,
                            kind="ExternalOutput").ap()

    xchunks = _x_chunks(levels, off, widths, NPC)

    with TileContext(nc) as tc:
        with tc.tile_pool(name="consts", bufs=1) as consts, \
             tc.tile_pool(name="keep", bufs=1) as keep, \
             tc.tile_pool(name="work", bufs=3) as work, \
             tc.tile_pool(name="xin", bufs=3) as xin, \
             tc.tile_pool(name="hout", bufs=3) as hout, \
             tc.tile_pool(name="psum", bufs=2, space="PSUM") as psum:

            wxb_s = consts.tile([INPUT + 1, 4 * H], BF16, name="wxb_s")
            nc.sync.dma_start(out=wxb_s, in_=wxb)
            whlr_s = consts.tile([H, 4 * H], BF16, name="whlr_s")
            nc.sync.dma_start(out=whlr_s, in_=whlr)

            # keep-state ping-pong, sized for the two largest parent levels
            khA = keep.tile([H, n_leaf // 2], BF16, name="khA")
            kcA = keep.tile([H, n_leaf // 2], BF16, name="kcA")
            khB = keep.tile([H, max(n_leaf // 4, 1)], BF16, name="khB")
            kcB = keep.tile([H, max(n_leaf // 4, 1)], BF16, name="kcB")

            def keep_bufs(l):
                """Tiles level l's stash writes (arranged for parent l-1)."""
                return (khA, kcA) if (levels[0] - l) % 2 == 0 else (khB, kcB)

            for _rep in range(repeats):
                xi = -1          # current x chunk index
                xt_ch = None     # current x chunk tile
                hst = None       # current h staging tile
                hst_base = hst_end = 0

                for l in levels:
                    n = widths[l]
                    leaf = l == levels[0]
                    half = n // 2
                    kh_t, kc_t = keep_bufs(l)
                    if leaf:
                        kh_p = kc_p = None
                    else:
                        kh_p, kc_p = keep_bufs(l + 1)
                    ntiles = (n + T - 1) // T

                    def resolve_x(cols, nt):
                        nonlocal xi, xt_ch
                        if xi < 0 or cols >= xchunks[xi][0] + xchunks[xi][1]:
                            xi += 1
                            cb, cw = xchunks[xi]
                            xt_ch = xin.tile([INPUT + 1, XCHUNK], BF16,
                                             tag="xt", name="xt")
                            nc.sync.dma_start(out=xt_ch[:, :cw],
                                              in_=xT[:, cb:cb + cw])
                        xb = cols - xchunks[xi][0]
                        return xt_ch[:, xb:xb + nt]

                    def resolve_hst(cols):
                        nonlocal hst, hst_base, hst_end
                        if hst is None or cols >= hst_end:
                            if hst is not None:
                                nc.sync.dma_start(
                                    out=out_hT[:, hst_base:hst_end],
                                    in_=hst[:, :hst_end - hst_base])
                            hst_base = cols
                            hst_end = min(cols + OCHUNK, NPC)
                            hst = hout.tile([H, OCHUNK], BF16, tag="hst",
                                            name="hst")
                        return hst, cols - hst_base

                    def emit_mms(pts, xts, kcs, nts):
                        """Gate matmuls for 1-2 tiles, stationaries paired."""
                        banks = (((0, GI), (1, GO), (2, GG)) if leaf else
                                 ((0, GI), (1, GF), (2, GO), (3, GG)))
                        for bank, g in banks:
                            for pt, xt, nt in zip(pts, xts, nts):
                                nc.tensor.matmul(
                                    pt[:, bank * T:bank * T + nt],
                                    wxb_s[:, g * H:(g + 1) * H],
                                    xt, start=True, stop=leaf)
                            if not leaf:
                                for pt, kc0, nt in zip(pts, kcs, nts):
                                    nc.tensor.matmul(
                                        pt[:, bank * T:bank * T + nt],
                                        whlr_s[:, g * H:(g + 1) * H],
                                        kh_p[:, kc0:kc0 + nt],
                                        start=False, stop=True)

                    def emit_body(pt, nt, kc0, hstv, ho):
                        """Activations + cell/h + stash for one tile."""
                        ng = 3 if leaf else 4
                        S = work.tile([H, 4 * T], BF16, tag="S", bufs=4,
                                      name="S")
                        nc.scalar.activation(
                            out=S[:, 0:ng * T].rearrange(
                                "p (g n) -> p g n", g=ng)[:, :, :nt],
                            in_=pt[:, 0:ng * T].rearrange(
                                "p (g n) -> p g n", g=ng)[:, :, :nt],
                            func=mybir.ActivationFunctionType.Sigmoid)
                        if leaf:
                            si = S[:, 0:nt]
                            so = S[:, T:T + nt]
                            tg = S[:, 2 * T:2 * T + nt]
                        else:
                            si = S[:, 0:nt]
                            sf = S[:, T:T + nt]
                            so = S[:, 2 * T:2 * T + nt]
                            tg = S[:, 3 * T:3 * T + nt]
                        # host doubled g weights: tanh(g) = 2*sigmoid(2g)-1
                        nc.vector.tensor_scalar(
                            out=tg, in0=tg, scalar1=2.0, scalar2=-1.0,
                            op0=mybir.AluOpType.mult, op1=mybir.AluOpType.add)

                        c = work.tile([H, T], BF16, tag="c", name="c")
                        if leaf:
                            nc.vector.tensor_mul(c[:, :nt], si, tg)
                        else:
                            t1 = work.tile([H, T], BF16, tag="t1", name="t1")
                            nc.vector.tensor_mul(t1[:, :nt], si, tg)
                            t2 = work.tile([H, T], BF16, tag="t2", name="t2")
                            # Pool is slow (~1.1us/tile); keep it off the
                            # latency-critical narrow tail levels
                            t2_eng = nc.gpsimd if n >= 2048 else nc.vector
                            t2_eng.tensor_mul(t2[:, :nt], sf,
                                              kc_p[:, kc0:kc0 + nt])
                            nc.vector.tensor_add(c[:, :nt], t1[:, :nt],
                                                 t2[:, :nt])

                        tch = work.tile([H, T], BF16, tag="tch", name="tch")
                        nc.scalar.activation(
                            out=tch[:, :nt], in_=c[:, :nt],
                            func=mybir.ActivationFunctionType.Tanh)
                        nc.vector.tensor_mul(hstv[:, ho:ho + nt], so,
                                             tch[:, :nt])

                        # stash h/c halves for the parent level:
                        # left children -> rows 0:64, right -> rows 64:128
                        a, b2 = kc0, kc0 + nt
                        segs = []
                        if a < half:
                            e = min(b2, half)
                            segs.append((0, a, a, e - a))
                        if b2 > half:
                            s0 = max(a, half)
                            segs.append((HH, s0 - half, s0, b2 - s0))
                        for r0, pc, sc, w in segs:
                            nc.vector.tensor_copy(
                                out=kh_t[r0:r0 + HH, pc:pc + w],
                                in_=hstv[0:HH, ho + sc - kc0:
                                         ho + sc - kc0 + w])
                            nc.vector.tensor_copy(
                                out=kc_t[r0:r0 + HH, pc:pc + w],
                                in_=c[0:HH, sc - kc0:sc - kc0 + w])

                    pos = 0
                    for grp in _tile_order(ntiles):
                        metas = []   # (nt, xcols, kc0)
                        for t in grp:
                            nt = min(T, n - t * T)
                            metas.append((nt, off[l] + pos * T, t * T))
                            pos += 1
                        xts = [resolve_x(cols, nt) for nt, cols, _ in metas]
                        hs = [resolve_hst(cols) for _, cols, _ in metas]
                        pts = [psum.tile([H, 4 * T], F32, tag="pt",
                                         name="pt") for _ in grp]
                        emit_mms(pts, xts, [m[2] for m in metas],
                                 [m[0] for m in metas])
                        for j in range(len(grp)):
                            nt, cols, kc0 = metas[j]
                            emit_body(pts[j], nt, kc0, hs[j][0], hs[j][1])

                # flush the last staging chunk
                nc.sync.dma_start(out=out_hT[:, hst_base:hst_end],
                                  in_=hst[:, :hst_end - hst_base])
                # dump level-DEV_MIN h/c halves (parent-arranged) for the host
                kh_t, kc_t = keep_bufs(levels[-1])
                nc.sync.dma_start(out=out_kh, in_=kh_t[:, 0:w_last // 2])
                nc.sync.dma_start(out=out_kc, in_=kc_t[:, 0:w_last // 2])

    nc.compile()
    return nc


_PROGRAMS = {}


def _get_program(L=LEVELS):
    if L not in _PROGRAMS:
        _PROGRAMS[L] = build_program(L)
    return _PROGRAMS[L]


def _prep_weights(W_ih, W_hh, b_ih, b_hh):
    import ml_dtypes
    b = (b_ih + b_hh).astype(np.float32)

    # permute gate blocks from [i, f, g, o] (torch order) to [i, f, o, g]
    def gperm(m):
        return np.concatenate(
            [m[0:H], m[H:2 * H], m[3 * H:4 * H], m[2 * H:3 * H]], axis=0)

    Wx = gperm(W_ih).copy()              # [512, 64]
    Wh = gperm(W_hh).copy()              # [512, 128]
    bp = gperm(b[:, None])[:, 0].copy()  # [512]
    # tanh(g) computed as 2*sigmoid(2g)-1 on device: double g's weights
    Wx[3 * H:4 * H] *= 2.0
    Wh[3 * H:4 * H] *= 2.0
    bp[3 * H:4 * H] *= 2.0

    wxb = np.concatenate([Wx.T, bp[None, :]], axis=0)       # [65, 512]
    # rows 0:64 apply to left-child h, rows 64:128 to right-child h
    whlr = np.concatenate([Wh[:, :HH].T, Wh[:, HH:].T], axis=0)  # [128, 512]
    return (wxb.astype(ml_dtypes.bfloat16),
            whlr.astype(ml_dtypes.bfloat16))


def _col_perms(levels, widths):
    """Per level: global-chunk index for each xT/out_hT column position."""
    perms = {}
    for l in levels:
        n = widths[l]
        perms[l] = _bitrev_perm(n)[_pos_perm(n)]
    return perms


def _make_in_maps(x, W_ih, W_hh, b_ih, b_hh, L=LEVELS):
    import ml_dtypes
    levels, widths, off, NPC = _layout(L)
    wxb, whlr = _prep_weights(W_ih, W_hh, b_ih, b_hh)
    perms = _col_perms(levels, widths)

    in_maps = []
    for k in range(N_CORES):
        xTk = np.empty((INPUT + 1, NPC), ml_dtypes.bfloat16)
        xTk[INPUT, :] = 1.0
        for l in levels:
            n = widths[l]
            start = 2 ** l - 1
            chunk = x[start + k * n: start + (k + 1) * n]  # [n, 64]
            xTk[:INPUT, off[l]:off[l] + n] = chunk[perms[l]].T
        in_maps.append({"xT": xTk, "wxb": wxb, "whlr": whlr})
    return in_maps, perms


def _assemble(results, x, W_ih, W_hh, b_ih, b_hh, perms, L=LEVELS):
    levels, widths, off, NPC = _layout(L)
    n_nodes = 2 ** L - 1
    out = np.zeros((n_nodes, H), np.float32)

    w_last = widths[levels[-1]]           # per-core level-DEV_MIN width
    n_last = w_last * N_CORES             # global level-DEV_MIN count
    h_half = np.zeros((n_last, HH), np.float32)
    c_half = np.zeros((n_last, HH), np.float32)
    bitrev_last = _bitrev_perm(w_last)

    for k in range(N_CORES):
        hk = np.asarray(results[k]["out_hT"]).astype(np.float32).T  # [NPC,128]
        for l in levels:
            n = widths[l]
            start = 2 ** l - 1
            out[start + k * n + perms[l]] = hk[off[l]:off[l] + n]
        kh = np.asarray(results[k]["out_kh"]).astype(np.float32)  # [128, w/2]
        kcv = np.asarray(results[k]["out_kc"]).astype(np.float32)
        # parent-arranged: col j holds left child (rows 0:64) = local col j,
        # right child (rows 64:128) = local col j + w_last//2
        hloc = np.empty((w_last, HH), np.float32)
        cloc = np.empty((w_last, HH), np.float32)
        hw = w_last // 2
        hloc[:hw] = kh[0:HH].T
        hloc[hw:] = kh[HH:H].T
        cloc[:hw] = kcv[0:HH].T
        cloc[hw:] = kcv[HH:H].T
        h_half[k * w_last + bitrev_last] = hloc
        c_half[k * w_last + bitrev_last] = cloc

    # levels DEV_MIN-1 .. 0 on host, mirroring the reference exactly
    b = (b_ih + b_hh).astype(np.float32)

    def sig(v):
        return 1.0 / (1.0 + np.exp(-v))

    hh_prev, cc_prev = h_half, c_half  # halves of the child level, in order
    for lvl in range(DEV_MIN - 1, -1, -1):
        start = 2 ** lvl - 1
        count = 2 ** lvl
        xs = x[start:start + count]
        h_prev = np.concatenate([hh_prev[0::2], hh_prev[1::2]], axis=-1)
        c_prev = np.concatenate([cc_prev[0::2], cc_prev[1::2]], axis=-1)
        gates = xs @ W_ih.T + h_prev @ W_hh.T + b
        gi, gf, gg, go = np.split(gates, 4, axis=-1)
        c = sig(gf) * c_prev + sig(gi) * np.tanh(gg)
        h = sig(go) * np.tanh(c)
        out[start:start + count] = h
        hh_prev, cc_prev = h[:, :HH], c[:, :HH]
    return out


def kernel(x, W_ih, W_hh, b_ih, b_hh):
    x = np.asarray(x, np.float32)
    W_ih = np.asarray(W_ih, np.float32)
    W_hh = np.asarray(W_hh, np.float32)
    b_ih = np.asarray(b_ih, np.float32)
    b_hh = np.asarray(b_hh, np.float32)

    nc = _get_program(LEVELS)
    in_maps, perms = _make_in_maps(x, W_ih, W_hh, b_ih, b_hh, LEVELS)
    res = None
    for attempt in range(3):
        try:
            res = run_bass_kernel_spmd(nc, in_maps,
                                       core_ids=list(range(N_CORES)))
            break
        except Exception:
            # transient device wedge (e.g. NRT_EXEC_UNIT_UNRECOVERABLE);
            # give the runtime a moment and retry
            if attempt == 2:
                raise
            import time as _time
            _time.sleep(10)
    return _assemble(res.results, x, W_ih, W_hh, b_ih, b_hh, perms, LEVELS)
